# revision 8
# baseline (speedup 1.0000x reference)
"""LoftQ linear (4-bit blockwise dequant + linear + LoRA) on 8 trn2 cores.

out = x @ W^T + bias + 2.0 * (x @ A^T) @ B^T
  W[o,i] = (idx[o,i] * 2/15 - 1) * scales[o, i//64]   (idx = 4-bit nibbles)

Sharding: column-parallel - qweight/scales/bias/lora_B sharded along
out_features (4096 -> 512 per core); x and lora_A replicated; outputs
concatenated on host.

V2 design notes (from trace analysis of V1 @ ~171us):
  - PE matmul stream is at roofline (216 ns / N=512 bf16 MM); all loss was
    DMA scheduling: first weight chunk landed at 19us and x chunks landed
    late (19us PE gap at 51us). Fabric sustains ~430 GB/s.
  - All INPUT DMAs go on the sync HWDGE ring in exact consumption order
    (ring drains FIFO in trace order); outputs+bias alone on the scalar
    ring so out-stores never delay x loads.
  - Input bytes cut 24.3 -> 21.6 MB: lora fold (2BA)^T shipped as
    fp8-e5m2 (|ba|~0.003 << |W|~0.35, error negligible), outputs bf16.
  - Pairs 0-1 shipped pre-dequantized (bf16) so real MMs start ~12us with
    zero dequant latency; 12 dummy warmup MMs cover the preamble window
    and HAM warm-up.
  - Pair-major MM order for ALL t-chunks (uniform CHUNKS-permuted x
    layout) so partially-arrived x unlocks MMs progressively; 4 psum
    banks per t-chunk, 8 total, stores on ScalarE (activation+bias,
    psum->sbuf bf16) overlapping next chunk's MMs.
"""

import numpy as np
import ml_dtypes

OUT_F = 4096
IN_F = 4096
T = 2048  # 2*1024 tokens
R = 16
NCORES = 8
O_SH = OUT_F // NCORES  # 512
IPH = IN_F // 2  # 2048 packed byte-rows
C16 = 2.0 / 15.0
NQ = IPH // 128  # 16 pairs
NI = IN_F // 128  # 32 i' chunks
NO = O_SH // 128  # 4 o tiles
NT = T // 512  # 4 t chunks
NWD = 2  # pairs shipped pre-dequantized
NLH = NQ - NWD  # 14 quantized pairs on device

BF16 = ml_dtypes.bfloat16
F8E5 = ml_dtypes.float8_e5m2
FP16 = np.float16

# x-position permutation: fully pair-interleaved — pos(k, lo)=2k,
# pos(k, hi)=2k+1, so x pieces of any granularity unlock in pair order
CHUNKS = [(0, 2), (2, 2), (4, 4), (8, 4), (12, 4)]
X0_ORDER = []
for _k in range(NQ):
    X0_ORDER += [_k, NQ + _k]
POS = {ic: i for i, ic in enumerate(X0_ORDER)}
# chunk-0 piece boundaries (start, len) in permuted positions: 2 pairs each
XG = [(4 * g, 4) for g in range(NQ // 2)]

_cached = {}


def _build_nc():
    import concourse.bacc as bacc
    import concourse.mybir as mybir
    from concourse.tile import TileContext

    f32 = mybir.dt.float32
    bf16 = mybir.dt.bfloat16
    fp16 = mybir.dt.float16
    f8e5 = mybir.dt.float8e5
    u8 = mybir.dt.uint8
    AF = mybir.ActivationFunctionType
    OP = mybir.AluOpType

    nc = bacc.Bacc("TRN2", target_bir_lowering=False)

    xt = nc.dram_tensor("xt", [128, NT, NI, 512], bf16, kind="ExternalInput")
    wd = nc.dram_tensor("wd", [128, NWD, 2 * O_SH], bf16, kind="ExternalInput")
    lh = nc.dram_tensor("lh", [128, NLH, 2 * O_SH], u8, kind="ExternalInput")
    sc = nc.dram_tensor("sc", [128, NLH, O_SH], fp16, kind="ExternalInput")
    ba = nc.dram_tensor("ba", [128, NLH, 2 * O_SH], f8e5, kind="ExternalInput")
    bias = nc.dram_tensor("bias", [O_SH, 1], f32, kind="ExternalInput")
    out = nc.dram_tensor("out", [O_SH, T], bf16, kind="ExternalOutput")

    with TileContext(nc) as tc:
        with (
            tc.tile_pool(name="w", bufs=1) as wpool,
            tc.tile_pool(name="x", bufs=1) as xpool,
            tc.tile_pool(name="xb", bufs=1) as xbpool,
            tc.tile_pool(name="wch", bufs=2) as wchpool,
            tc.tile_pool(name="cst", bufs=1) as cpool,
            tc.tile_pool(name="dq", bufs=2) as dqpool,
            tc.tile_pool(name="outp", bufs=3) as opool,
            tc.tile_pool(name="ps", bufs=8, space="PSUM") as pspool,
        ):
            # bias (scalar ring; tiny, out of the input queue's way)
            bias_sb = []
            for ot in range(NO):
                btile = cpool.tile([128, 1], f32, tag=f"bias{ot}", name=f"biassb{ot}")
                nc.scalar.dma_start(
                    out=btile[:], in_=bias[ot * 128 : (ot + 1) * 128, :]
                )
                bias_sb.append(btile)

            # PE warm-up: dummy matmuls so the HAM clock gate opens and the
            # PE has work while the first real inputs stream in
            wsc = cpool.tile([128, 512], bf16, tag="wsc", name="wsc")
            nc.vector.memset(wsc[:], 0)
            psc = pspool.tile([128, 512], f32, tag="mm", name="psc")
            NWARM = 14
            for d in range(NWARM):
                nc.tensor.matmul(
                    psc[:], wsc[:, :128], wsc[:],
                    start=(d == 0), stop=(d == NWARM - 1),
                )

            Wp = [
                wpool.tile([128, 2 * O_SH], bf16, tag=f"w{k}", name=f"wt{k}")
                for k in range(NQ)
            ]

            # ---- the ordered input queue (sync ring, FIFO in trace order).
            # Pre-dequantized head pairs, then x/W chunks interleaved in
            # exact consumption order, then the later t-chunks.
            nc.sync.dma_start(out=Wp[0][:], in_=wd[:, 0, :])
            nc.sync.dma_start(out=Wp[1][:], in_=wd[:, 1, :])

            def wchunk(ci, k0, np_):
                """DMA one W chunk (lh/sc/ba) + its dequant chain.

                The fp8 ba is upconverted to fp16 on the otherwise-idle
                GpSimd engine (1-input copy = line rate) so the DVE add
                runs in 2x mode — DVE per pair ~1.1us, under the 1.73us
                MM consumption rate."""
                r0 = k0 - NWD
                lt = wchpool.tile([128, np_, 2 * O_SH], u8, tag="lhc", name=f"lhc{ci}")
                nc.sync.dma_start(out=lt[:], in_=lh[:, r0 : r0 + np_])
                st2 = wchpool.tile([128, np_, O_SH], fp16, tag="scc", name=f"scc{ci}")
                nc.sync.dma_start(out=st2[:], in_=sc[:, r0 : r0 + np_])
                bt = wchpool.tile([128, np_, 2 * O_SH], f8e5, tag="bac", name=f"bac{ci}")
                nc.sync.dma_start(out=bt[:], in_=ba[:, r0 : r0 + np_])
                for j in range(np_):
                    k = k0 + j
                    upf = dqpool.tile([128, 2 * O_SH], fp16, tag="upf", name=f"upf{k}")
                    nc.scalar.activation(
                        upf[:], lt[:, j, :], AF.Copy, bias=-1.0, scale=C16
                    )
                    ba16 = dqpool.tile([128, 2 * O_SH], fp16, tag="ba16", name=f"ba16_{k}")
                    nc.gpsimd.tensor_copy(ba16[:], bt[:, j, :])
                    nc.vector.tensor_tensor(
                        Wp[k][:],
                        upf[:],
                        st2[:, j, :][:, None, :].to_broadcast([128, 2, O_SH]),
                        OP.mult,
                    )
                    nc.vector.tensor_tensor(Wp[k][:], Wp[k][:], ba16[:], OP.add)

            # 2-pair W chunks, each just before the x piece of its pairs —
            # smooth pair-paced supply for both W and x
            x0t = []  # chunk-0 pieces, 2 pairs each
            for gi, (st_, ln) in enumerate(XG):
                if gi >= 1:
                    wchunk(gi, 2 * gi, 2)
                xa = xpool.tile([128, ln, 512], bf16, tag=f"xa{gi}", name=f"xa{gi}")
                x0t.append(xa)
                nc.sync.dma_start(out=xa[:], in_=xt[:, 0, st_ : st_ + ln])

            # later t-chunks: two halves for t1, one whole tile for t2, and
            # t3 halves rotated into t1's slots (WAR deps self-pace: xh0's
            # last reader is mid-t1, long before t3 needs data)
            xh = []
            for hi in range(2):
                xbt = xbpool.tile([128, 16, 512], bf16, tag=f"xh{hi}", name=f"xh{hi}")
                nc.sync.dma_start(out=xbt[:], in_=xt[:, 1, hi * 16 : (hi + 1) * 16])
                xh.append(xbt)
            xc2t = xbpool.tile([128, NI, 512], bf16, tag="xbig", name="xc2")
            nc.sync.dma_start(out=xc2t[:], in_=xt[:, 2])
            x3h = []
            for hi in range(2):
                xbt = xbpool.tile(
                    [128, 16, 512], bf16, tag=f"xh{hi}", name=f"x3h{hi}"
                )
                nc.sync.dma_start(out=xbt[:], in_=xt[:, 3, hi * 16 : (hi + 1) * 16])
                x3h.append(xbt)

            def xsrc(tcn, k, half):
                pos = POS[k + half * NQ]
                if tcn == 0:
                    for gi, (st_, ln) in enumerate(XG):
                        if st_ <= pos < st_ + ln:
                            return x0t[gi][:, pos - st_, :]
                if tcn == 1:
                    return xh[pos // 16][:, pos % 16, :]
                if tcn == 2:
                    return xc2t[:, pos, :]
                return x3h[pos // 16][:, pos % 16, :]

            # ---- main matmuls; stores on ScalarE (activation + bias).
            # t0-t2 pair-major (tolerates streaming x); t3 ot-major so the
            # first 3 stores overlap remaining MMs and only the last store
            # is exposed in the tail.
            def store(p, tcn, ot):
                o_sb = opool.tile([128, 512], bf16, tag="osb", name=f"osb{tcn}_{ot}")
                nc.scalar.activation(
                    o_sb[:], p[:], AF.Identity, bias=bias_sb[ot][:], scale=1.0
                )
                nc.scalar.dma_start(
                    out=out[ot * 128 : (ot + 1) * 128, tcn * 512 : (tcn + 1) * 512],
                    in_=o_sb[:],
                )

            for tcn in range(NT - 1):
                p = [
                    pspool.tile([128, 512], f32, tag="mm", name=f"p{tcn}_{ot}")
                    for ot in range(NO)
                ]
                for k in range(NQ):
                    for half in range(2):
                        xs = xsrc(tcn, k, half)
                        for ot in range(NO):
                            nc.tensor.matmul(
                                p[ot][:],
                                Wp[k][
                                    :,
                                    half * O_SH + ot * 128 : half * O_SH + (ot + 1) * 128,
                                ],
                                xs,
                                start=(k == 0 and half == 0),
                                stop=(k == NQ - 1 and half == 1),
                            )
                for ot in range(NO):
                    store(p[ot], tcn, ot)

            tcn = NT - 1
            for ot in range(NO):
                p = pspool.tile([128, 512], f32, tag="mm", name=f"p{tcn}_{ot}")
                n = 0
                for k in range(NQ):
                    for half in range(2):
                        nc.tensor.matmul(
                            p[:],
                            Wp[k][
                                :,
                                half * O_SH + ot * 128 : half * O_SH + (ot + 1) * 128,
                            ],
                            xsrc(tcn, k, half),
                            start=(n == 0),
                            stop=(n == 2 * NQ - 1),
                        )
                        n += 1
                store(p, tcn, ot)
    nc.compile()
    return nc


def _pack_rows(a, nblk):
    """[nblk*128, F] -> [128, nblk, F] with blk j, partition p = row j*128+p."""
    f = a.shape[1]
    return np.ascontiguousarray(a.reshape(nblk, 128, f).transpose(1, 0, 2))


def prep_inputs(x, qweight, scales, bias, lora_A, lora_B):
    """Host-side layout prep + sharding. Returns per-core input maps."""
    x2d = np.ascontiguousarray(x.reshape(T, IN_F))
    xtr = x2d.T  # [IN_F, T]
    # i' permutation: even original i first, then odd
    xp = np.concatenate([xtr[0::2], xtr[1::2]], axis=0)
    xb = _pack_rows(xp, NI)  # [128, NI, T]
    xb = np.ascontiguousarray(
        xb.reshape(128, NI, NT, 512).transpose(0, 2, 1, 3)
    )  # [128, NT, NI, 512]
    xtp = np.ascontiguousarray(xb[:, :, X0_ORDER, :]).astype(BF16)

    ap = np.ascontiguousarray(
        np.concatenate([lora_A[:, 0::2], lora_A[:, 1::2]], axis=1)
    ).astype(np.float32)  # [R, IN_F] permuted

    qw2 = qweight.reshape(OUT_F, IPH)  # byte (o, ip) holds i=2ip (lo), 2ip+1 (hi)
    sc2 = scales.reshape(OUT_F, IN_F // 64)

    in_maps = []
    for c in range(NCORES):
        o0, o1 = c * O_SH, (c + 1) * O_SH
        qp = _pack_rows(qw2[o0:o1].T, NQ)  # [128, NQ, O_SH] packed bytes
        lo = (qp & 15).astype(np.float32)
        hi = ((qp >> 4) & 15).astype(np.float32)
        # scale for (ip, o) = scales[o, ip//32] (same for lo and hi nibble)
        st_c = _pack_rows(
            np.repeat(sc2[o0:o1].T, 32, axis=0).astype(np.float32), NQ
        )  # [128, NQ, O_SH]
        ba3 = _pack_rows(
            (ap.T @ (2.0 * lora_B[o0:o1].T)).astype(np.float32), NI
        )  # [128, NI, O_SH]
        ba_pair = np.concatenate(
            [ba3[:, :NQ, :], ba3[:, NQ:, :]], axis=2
        )  # [128, NQ, 2*O_SH]

        # head pairs fully dequantized on host (bf16, ready for matmul)
        wfull = np.concatenate(
            [(lo * C16 - 1.0) * st_c, (hi * C16 - 1.0) * st_c], axis=2
        )
        # device path rounds the scaled value to bf16 then adds the fp8 ba
        # in bf16; mirror roughly by computing in f32 (tolerance is loose)
        wd_c = np.ascontiguousarray(
            (wfull + ba_pair)[:, :NWD, :]
        ).astype(BF16)

        lh_c = np.ascontiguousarray(
            np.concatenate(
                [qp[:, NWD:, :] & 15, (qp[:, NWD:, :] >> 4) & 15], axis=2
            )
        ).astype(np.uint8)  # [128, NLH, 2*O_SH] nibbles
        sc_c = np.ascontiguousarray(st_c[:, NWD:, :]).astype(FP16)
        ba_c = np.ascontiguousarray(ba_pair[:, NWD:, :]).astype(F8E5)
        bias_c = np.ascontiguousarray(bias[o0:o1].reshape(O_SH, 1)).astype(np.float32)
        in_maps.append(
            {
                "xt": xtp,
                "wd": wd_c,
                "lh": lh_c,
                "sc": sc_c,
                "ba": ba_c,
                "bias": bias_c,
            }
        )
    return in_maps


def run(in_maps, trace=False):
    from concourse import bass_utils

    if "nc" not in _cached:
        _cached["nc"] = _build_nc()
    res = bass_utils.run_bass_kernel_spmd(
        _cached["nc"], in_maps, list(range(NCORES)), trace=trace
    )
    return res


def assemble(results):
    full = np.concatenate(
        [np.asarray(r["out"]).astype(np.float32) for r in results], axis=0
    )  # [OUT_F, T]
    return np.ascontiguousarray(full.T).reshape(2, 1024, OUT_F)


def kernel(x, qweight, scales, bias, lora_A, lora_B):
    in_maps = prep_inputs(x, qweight, scales, bias, lora_A, lora_B)
    res = run(in_maps, trace=False)
    return assemble(res.results)


# revision 29
# speedup vs baseline: 1.0713x; 1.0713x over previous
"""LoftQ linear (4-bit blockwise dequant + linear + LoRA) on 8 trn2 cores.

out = x @ W^T + bias + 2.0 * (x @ A^T) @ B^T
  W[o,i] = (idx[o,i] * 2/15 - 1) * scales[o, i//64]   (idx = 4-bit nibbles)

Sharding: column-parallel - qweight/scales/bias/lora_B sharded along
out_features (4096 -> 512 per core); x and lora_A replicated; outputs
concatenated on host.

Design notes (from trace analysis; baseline ~171us -> ~135us):
  - PE matmul stream is at roofline (216 ns / N=512 bf16 MM, 512 MMs =
    110.6us/core); all loss was DMA scheduling. Fabric sustains ~430 GB/s
    but transfers only start ~8.6us (framework preamble).
  - All INPUT DMAs go on the sync HWDGE ring in exact consumption order
    (per-engine rings drain FIFO in trace order); outputs+bias alone on
    the scalar ring so out-stores never delay x loads.
  - Input bytes cut 24.3 -> 21.6 MB: lora fold (2BA)^T shipped as
    fp8-e5m2 (|ba|~0.003 << |W|~0.35, error negligible), outputs bf16.
  - Pairs 0-1 shipped pre-dequantized (bf16) so real MMs start ~11us with
    zero dequant latency; 8 dummy warmup MMs (~= the 3.4us HAM window)
    bridge the preamble and open the PE clock gate.
  - x layout pair-interleaved (pos 2k = pair-k lo, 2k+1 = hi); 2-pair W
    chunks each DMA'd just before their x piece, so W/x supply is pair-
    paced and partially-arrived data unlocks MMs progressively.
  - Pair-major MM order for t0-t2 (4 psum banks/chunk, 8 total, stores on
    ScalarE activation+bias overlapping the next chunk); t3 ot-major so
    only its last store is exposed in the tail.
  - Known-bad variants (measured): GpSimd fp8 CAST (3.7us software loop,
    shared-port lock starves DVE); stride-0 broadcast-DMA for compact
    scales (descriptor explosion blocks the HWDGE rings); fp16 ba/s
    reassociation (DVE 2x win < extra DMA in the critical window).
"""

import numpy as np
import ml_dtypes

OUT_F = 4096
IN_F = 4096
T = 2048  # 2*1024 tokens
R = 16
NCORES = 8
O_SH = OUT_F // NCORES  # 512
IPH = IN_F // 2  # 2048 packed byte-rows
C16 = 2.0 / 15.0
NQ = IPH // 128  # 16 pairs
NI = IN_F // 128  # 32 i' chunks
NO = O_SH // 128  # 4 o tiles
NT = T // 512  # 4 t chunks
NWD = 2  # pairs shipped pre-dequantized
NLH = NQ - NWD  # 14 quantized pairs on device

BF16 = ml_dtypes.bfloat16
F8E5 = ml_dtypes.float8_e5m2
FP16 = np.float16

# x-position permutation: fully pair-interleaved — pos(k, lo)=2k,
# pos(k, hi)=2k+1, so x pieces of any granularity unlock in pair order
X0_ORDER = []
for _k in range(NQ):
    X0_ORDER += [_k, NQ + _k]
POS = {ic: i for i, ic in enumerate(X0_ORDER)}
# chunk-0 piece boundaries (start, len) in permuted positions: 2 pairs each
XG = [(4 * g, 4) for g in range(NQ // 2)]

_cached = {}


def _build_nc():
    import concourse.bacc as bacc
    import concourse.mybir as mybir
    from concourse.tile import TileContext

    f32 = mybir.dt.float32
    bf16 = mybir.dt.bfloat16
    fp16 = mybir.dt.float16
    f8e5 = mybir.dt.float8e5
    u8 = mybir.dt.uint8
    AF = mybir.ActivationFunctionType
    OP = mybir.AluOpType

    nc = bacc.Bacc("TRN2", target_bir_lowering=False)

    xt = nc.dram_tensor("xt", [128, NT, NI, 512], bf16, kind="ExternalInput")
    wd = nc.dram_tensor("wd", [128, NWD, 2 * O_SH], bf16, kind="ExternalInput")
    lh = nc.dram_tensor("lh", [128, NLH, 2 * O_SH], u8, kind="ExternalInput")
    # scales shipped fully replicated (fp16). A compact-scales variant with
    # on-chip SBUF->SBUF broadcast DMA was tried and reverted: the stride-0
    # source AP explodes into ~128 tiny descriptors per pair, fills the
    # HWDGE rings, and blocks DMA issue (dequant crawled at 4.3us/pair).
    sc = nc.dram_tensor("sc", [128, NLH, O_SH], fp16, kind="ExternalInput")
    # lora fold in fp8-e5m2: the DVE add runs 1x (~1.2us/pair) but the
    # halved DMA footprint in the critical t0 window wins over a fp16
    # ba/s variant (measured: 8.9us vs 11us of warm-clock PE gaps)
    ba = nc.dram_tensor("ba", [128, NLH, 2 * O_SH], f8e5, kind="ExternalInput")
    bias = nc.dram_tensor("bias", [O_SH, 1], f32, kind="ExternalInput")
    out = nc.dram_tensor("out", [O_SH, T], bf16, kind="ExternalOutput")

    with TileContext(nc) as tc:
        with (
            tc.tile_pool(name="w", bufs=1) as wpool,
            tc.tile_pool(name="x", bufs=1) as xpool,
            tc.tile_pool(name="xb", bufs=1) as xbpool,
            tc.tile_pool(name="wch", bufs=2) as wchpool,
            tc.tile_pool(name="cst", bufs=1) as cpool,
            tc.tile_pool(name="dq", bufs=2) as dqpool,
            tc.tile_pool(name="outp", bufs=3) as opool,
            tc.tile_pool(name="ps", bufs=8, space="PSUM") as pspool,
        ):
            # bias (scalar ring; tiny, out of the input queue's way)
            bias_sb = []
            for ot in range(NO):
                btile = cpool.tile([128, 1], f32, tag=f"bias{ot}", name=f"biassb{ot}")
                nc.scalar.dma_start(
                    out=btile[:], in_=bias[ot * 128 : (ot + 1) * 128, :]
                )
                bias_sb.append(btile)

            # PE warm-up: dummy matmuls so the HAM clock gate opens while
            # inputs stream in. memset on GpSimd (idle; clears its preamble
            # earliest); 8 cold MMs ~= the 3.4us HAM window, short enough
            # not to delay real MMs when the preamble runs late.
            wsc = cpool.tile([128, 512], bf16, tag="wsc", name="wsc")
            nc.gpsimd.memset(wsc[:], 0)
            psc = pspool.tile([128, 512], f32, tag="mm", name="psc")
            NWARM = 8
            for d in range(NWARM):
                nc.tensor.matmul(
                    psc[:], wsc[:, :128], wsc[:],
                    start=(d == 0), stop=(d == NWARM - 1),
                )

            Wp = [
                wpool.tile([128, 2 * O_SH], bf16, tag=f"w{k}", name=f"wt{k}")
                for k in range(NQ)
            ]

            # ---- the ordered input queue (sync ring, FIFO in trace order).
            # Pre-dequantized head pairs, then x/W chunks interleaved in
            # exact consumption order, then the later t-chunks.
            nc.sync.dma_start(out=Wp[0][:], in_=wd[:, 0, :])

            def wchunk(ci, k0, np_):
                """DMA one W chunk (lh/ba) + its dequant chain.

                DVE per pair: mult 2x (~0.55us) + fp8 add 1x (~1.1us)
                = 1.6us, just under the 1.73us/pair MM consumption rate.
                (A GpSimd fp8->fp16 upconvert was tried and reverted:
                GpSimd CAST is ~3.7us software loop and its shared-port
                lock starves concurrent DVE 2-port ops.)"""
                r0 = k0 - NWD
                lt = wchpool.tile([128, np_, 2 * O_SH], u8, tag="lhc", name=f"lhc{ci}")
                nc.sync.dma_start(out=lt[:], in_=lh[:, r0 : r0 + np_])
                st2 = wchpool.tile([128, np_, O_SH], fp16, tag="scc", name=f"scc{ci}")
                nc.sync.dma_start(out=st2[:], in_=sc[:, r0 : r0 + np_])
                bt = wchpool.tile([128, np_, 2 * O_SH], f8e5, tag="bac", name=f"bac{ci}")
                nc.sync.dma_start(out=bt[:], in_=ba[:, r0 : r0 + np_])
                for j in range(np_):
                    k = k0 + j
                    upf = dqpool.tile([128, 2 * O_SH], fp16, tag="upf", name=f"upf{k}")
                    nc.scalar.activation(
                        upf[:], lt[:, j, :], AF.Copy, bias=-1.0, scale=C16
                    )
                    nc.vector.tensor_tensor(
                        Wp[k][:],
                        upf[:],
                        st2[:, j, :][:, None, :].to_broadcast([128, 2, O_SH]),
                        OP.mult,
                    )
                    nc.vector.tensor_tensor(Wp[k][:], Wp[k][:], bt[:, j, :], OP.add)

            # 2-pair W chunks, each just before the x piece of its pairs —
            # smooth pair-paced supply for both W and x
            x0t = []  # chunk-0 pieces, 2 pairs each
            for gi, (st_, ln) in enumerate(XG):
                if gi == 1:
                    # second head pair after the first x piece, so MM
                    # (pair 0) starts ~1us earlier
                    nc.sync.dma_start(out=Wp[1][:], in_=wd[:, 1, :])
                if gi >= 1:
                    wchunk(gi, 2 * gi, 2)
                xa = xpool.tile([128, ln, 512], bf16, tag=f"xa{gi}", name=f"xa{gi}")
                x0t.append(xa)
                nc.sync.dma_start(out=xa[:], in_=xt[:, 0, st_ : st_ + ln])

            # later t-chunks: two halves for t1, one whole tile for t2, and
            # t3 halves rotated into t1's slots (WAR deps self-pace: xh0's
            # last reader is mid-t1, long before t3 needs data)
            xh = []
            for hi in range(2):
                xbt = xbpool.tile([128, 16, 512], bf16, tag=f"xh{hi}", name=f"xh{hi}")
                nc.sync.dma_start(out=xbt[:], in_=xt[:, 1, hi * 16 : (hi + 1) * 16])
                xh.append(xbt)
            xc2t = xbpool.tile([128, NI, 512], bf16, tag="xbig", name="xc2")
            nc.sync.dma_start(out=xc2t[:], in_=xt[:, 2])
            x3h = []
            for hi in range(2):
                xbt = xbpool.tile(
                    [128, 16, 512], bf16, tag=f"xh{hi}", name=f"x3h{hi}"
                )
                nc.sync.dma_start(out=xbt[:], in_=xt[:, 3, hi * 16 : (hi + 1) * 16])
                x3h.append(xbt)

            def xsrc(tcn, k, half):
                pos = POS[k + half * NQ]
                if tcn == 0:
                    for gi, (st_, ln) in enumerate(XG):
                        if st_ <= pos < st_ + ln:
                            return x0t[gi][:, pos - st_, :]
                if tcn == 1:
                    return xh[pos // 16][:, pos % 16, :]
                if tcn == 2:
                    return xc2t[:, pos, :]
                return x3h[pos // 16][:, pos % 16, :]

            # ---- main matmuls; stores on ScalarE (activation + bias).
            # t0-t2 pair-major (tolerates streaming x); t3 ot-major so the
            # first 3 stores overlap remaining MMs and only the last store
            # is exposed in the tail.
            def store(p, tcn, ot):
                o_sb = opool.tile([128, 512], bf16, tag="osb", name=f"osb{tcn}_{ot}")
                nc.scalar.activation(
                    o_sb[:], p[:], AF.Identity, bias=bias_sb[ot][:], scale=1.0
                )
                nc.scalar.dma_start(
                    out=out[ot * 128 : (ot + 1) * 128, tcn * 512 : (tcn + 1) * 512],
                    in_=o_sb[:],
                )

            for tcn in range(NT - 1):
                p = [
                    pspool.tile([128, 512], f32, tag="mm", name=f"p{tcn}_{ot}")
                    for ot in range(NO)
                ]
                for k in range(NQ):
                    for half in range(2):
                        xs = xsrc(tcn, k, half)
                        for ot in range(NO):
                            nc.tensor.matmul(
                                p[ot][:],
                                Wp[k][
                                    :,
                                    half * O_SH + ot * 128 : half * O_SH + (ot + 1) * 128,
                                ],
                                xs,
                                start=(k == 0 and half == 0),
                                stop=(k == NQ - 1 and half == 1),
                            )
                for ot in range(NO):
                    store(p[ot], tcn, ot)

            tcn = NT - 1
            for ot in range(NO):
                p = pspool.tile([128, 512], f32, tag="mm", name=f"p{tcn}_{ot}")
                n = 0
                for k in range(NQ):
                    for half in range(2):
                        nc.tensor.matmul(
                            p[:],
                            Wp[k][
                                :,
                                half * O_SH + ot * 128 : half * O_SH + (ot + 1) * 128,
                            ],
                            xsrc(tcn, k, half),
                            start=(n == 0),
                            stop=(n == 2 * NQ - 1),
                        )
                        n += 1
                store(p, tcn, ot)
    nc.compile()
    return nc


def _pack_rows(a, nblk):
    """[nblk*128, F] -> [128, nblk, F] with blk j, partition p = row j*128+p."""
    f = a.shape[1]
    return np.ascontiguousarray(a.reshape(nblk, 128, f).transpose(1, 0, 2))


def prep_inputs(x, qweight, scales, bias, lora_A, lora_B):
    """Host-side layout prep + sharding. Returns per-core input maps."""
    x2d = np.ascontiguousarray(x.reshape(T, IN_F))
    xtr = x2d.T  # [IN_F, T]
    # i' permutation: even original i first, then odd
    xp = np.concatenate([xtr[0::2], xtr[1::2]], axis=0)
    xb = _pack_rows(xp, NI)  # [128, NI, T]
    xb = np.ascontiguousarray(
        xb.reshape(128, NI, NT, 512).transpose(0, 2, 1, 3)
    )  # [128, NT, NI, 512]
    xtp = np.ascontiguousarray(xb[:, :, X0_ORDER, :]).astype(BF16)

    ap = np.ascontiguousarray(
        np.concatenate([lora_A[:, 0::2], lora_A[:, 1::2]], axis=1)
    ).astype(np.float32)  # [R, IN_F] permuted

    qw2 = qweight.reshape(OUT_F, IPH)  # byte (o, ip) holds i=2ip (lo), 2ip+1 (hi)
    sc2 = scales.reshape(OUT_F, IN_F // 64)

    in_maps = []
    for c in range(NCORES):
        o0, o1 = c * O_SH, (c + 1) * O_SH
        qp = _pack_rows(qw2[o0:o1].T, NQ)  # [128, NQ, O_SH] packed bytes
        lo = (qp & 15).astype(np.float32)
        hi = ((qp >> 4) & 15).astype(np.float32)
        # scale for (ip, o) = scales[o, ip//32] (same for lo and hi nibble)
        st_c = _pack_rows(
            np.repeat(sc2[o0:o1].T, 32, axis=0).astype(np.float32), NQ
        )  # [128, NQ, O_SH]
        ba3 = _pack_rows(
            (ap.T @ (2.0 * lora_B[o0:o1].T)).astype(np.float32), NI
        )  # [128, NI, O_SH]
        ba_pair = np.concatenate(
            [ba3[:, :NQ, :], ba3[:, NQ:, :]], axis=2
        )  # [128, NQ, 2*O_SH]

        # head pairs fully dequantized on host (bf16, ready for matmul)
        wfull = np.concatenate(
            [(lo * C16 - 1.0) * st_c, (hi * C16 - 1.0) * st_c], axis=2
        )
        # device path rounds the scaled value to bf16 then adds the fp8 ba
        # in bf16; mirror roughly by computing in f32 (tolerance is loose)
        wd_c = np.ascontiguousarray(
            (wfull + ba_pair)[:, :NWD, :]
        ).astype(BF16)

        lh_c = np.ascontiguousarray(
            np.concatenate(
                [qp[:, NWD:, :] & 15, (qp[:, NWD:, :] >> 4) & 15], axis=2
            )
        ).astype(np.uint8)  # [128, NLH, 2*O_SH] nibbles
        sc_c = np.ascontiguousarray(st_c[:, NWD:, :]).astype(FP16)
        ba_c = np.ascontiguousarray(ba_pair[:, NWD:, :]).astype(F8E5)
        bias_c = np.ascontiguousarray(bias[o0:o1].reshape(O_SH, 1)).astype(np.float32)
        in_maps.append(
            {
                "xt": xtp,
                "wd": wd_c,
                "lh": lh_c,
                "sc": sc_c,
                "ba": ba_c,
                "bias": bias_c,
            }
        )
    return in_maps


def run(in_maps, trace=False):
    from concourse import bass_utils

    if "nc" not in _cached:
        _cached["nc"] = _build_nc()
    res = bass_utils.run_bass_kernel_spmd(
        _cached["nc"], in_maps, list(range(NCORES)), trace=trace
    )
    return res


def assemble(results):
    full = np.concatenate(
        [np.asarray(r["out"]).astype(np.float32) for r in results], axis=0
    )  # [OUT_F, T]
    return np.ascontiguousarray(full.T).reshape(2, 1024, OUT_F)


def kernel(x, qweight, scales, bias, lora_A, lora_B):
    in_maps = prep_inputs(x, qweight, scales, bias, lora_A, lora_B)
    res = run(in_maps, trace=False)
    return assemble(res.results)


# revision 35
# speedup vs baseline: 1.2597x; 1.1758x over previous
"""LoftQ linear (4-bit blockwise dequant + linear + LoRA) on 8 trn2 cores.

out = x @ W^T + bias + 2.0 * (x @ A^T) @ B^T
  W[o,i] = (idx[o,i] * 2/15 - 1) * scales[o, i//64]   (idx = 4-bit nibbles)

Sharding: column-parallel - qweight/scales/bias/lora_B sharded along
out_features (4096 -> 512 per core); x and lora_A replicated; outputs
concatenated on host.

Design notes (from trace analysis; baseline ~171us -> ~135us):
  - PE matmul stream is at roofline (216 ns / N=512 bf16 MM, 512 MMs =
    110.6us/core); all loss was DMA scheduling. Fabric sustains ~430 GB/s
    but transfers only start ~8.6us (framework preamble).
  - All INPUT DMAs go on the sync HWDGE ring in exact consumption order
    (per-engine rings drain FIFO in trace order); outputs+bias alone on
    the scalar ring so out-stores never delay x loads.
  - Input bytes cut 24.3 -> 21.6 MB: lora fold (2BA)^T shipped as
    fp8-e5m2 (|ba|~0.003 << |W|~0.35, error negligible), outputs bf16.
  - Pairs 0-1 shipped pre-dequantized (bf16) so real MMs start ~11us with
    zero dequant latency; 8 dummy warmup MMs (~= the 3.4us HAM window)
    bridge the preamble and open the PE clock gate.
  - x layout pair-interleaved (pos 2k = pair-k lo, 2k+1 = hi); 2-pair W
    chunks each DMA'd just before their x piece, so W/x supply is pair-
    paced and partially-arrived data unlocks MMs progressively.
  - Pair-major MM order for t0-t2 (4 psum banks/chunk, 8 total, stores on
    ScalarE activation+bias overlapping the next chunk); t3 ot-major so
    only its last store is exposed in the tail.
  - Known-bad variants (measured): GpSimd fp8 CAST (3.7us software loop,
    shared-port lock starves DVE); stride-0 broadcast-DMA for compact
    scales (descriptor explosion blocks the HWDGE rings); fp16 ba/s
    reassociation (DVE 2x win < extra DMA in the critical window).
"""

import numpy as np
import ml_dtypes

OUT_F = 4096
IN_F = 4096
T = 2048  # 2*1024 tokens
R = 16
NCORES = 8
O_SH = OUT_F // NCORES  # 512
IPH = IN_F // 2  # 2048 packed byte-rows
C16 = 2.0 / 15.0
NQ = IPH // 128  # 16 pairs
NI = IN_F // 128  # 32 i' chunks
NO = O_SH // 128  # 4 o tiles
NT = T // 512  # 4 t chunks
NWD = 2  # pairs shipped pre-dequantized
NLH = NQ - NWD  # 14 quantized pairs on device

BF16 = ml_dtypes.bfloat16
F8E5 = ml_dtypes.float8_e5m2
FP16 = np.float16

# Pair consumption order: quantized pairs first, pre-dequantized head
# pairs LAST — the 14 DVE dequant chains (~1.9us each) run slightly
# slower than the 1.73us/pair MM demand, and the two zero-work wd slots
# at the tail absorb the accumulated slip.
PSEQ = list(range(NWD, NQ)) + list(range(NWD))
# x-position permutation: pair-interleaved in PSEQ order — each pair's
# lo/hi tiles adjacent, so x pieces of any granularity unlock in
# consumption order
X0_ORDER = []
for _k in PSEQ:
    X0_ORDER += [_k, NQ + _k]
POS = {ic: i for i, ic in enumerate(X0_ORDER)}
# chunk-0 piece boundaries (start, len) in permuted positions: 2 pairs each
XG = [(4 * g, 4) for g in range(NQ // 2)]

_cached = {}


def _build_nc():
    import concourse.bacc as bacc
    import concourse.mybir as mybir
    from concourse.tile import TileContext

    f32 = mybir.dt.float32
    bf16 = mybir.dt.bfloat16
    fp16 = mybir.dt.float16
    f8e5 = mybir.dt.float8e5
    u8 = mybir.dt.uint8
    AF = mybir.ActivationFunctionType
    OP = mybir.AluOpType

    nc = bacc.Bacc("TRN2", target_bir_lowering=False)

    xt = nc.dram_tensor("xt", [128, NT, NI, 512], bf16, kind="ExternalInput")
    wd = nc.dram_tensor("wd", [128, NWD, 2 * O_SH], bf16, kind="ExternalInput")
    lh = nc.dram_tensor("lh", [128, NLH, 2 * O_SH], u8, kind="ExternalInput")
    # scales shipped fully replicated (fp16). A compact-scales variant with
    # on-chip SBUF->SBUF broadcast DMA was tried and reverted: the stride-0
    # source AP explodes into ~128 tiny descriptors per pair, fills the
    # HWDGE rings, and blocks DMA issue (dequant crawled at 4.3us/pair).
    sc = nc.dram_tensor("sc", [128, NLH, O_SH], fp16, kind="ExternalInput")
    # lora fold in fp8-e5m2: the DVE add runs 1x (~1.2us/pair) but the
    # halved DMA footprint in the critical t0 window wins over a fp16
    # ba/s variant (measured: 8.9us vs 11us of warm-clock PE gaps)
    ba = nc.dram_tensor("ba", [128, NLH, 2 * O_SH], f8e5, kind="ExternalInput")
    bias = nc.dram_tensor("bias", [O_SH, 1], f32, kind="ExternalInput")
    out = nc.dram_tensor("out", [O_SH, T], bf16, kind="ExternalOutput")

    with TileContext(nc) as tc:
        with (
            tc.tile_pool(name="w", bufs=1) as wpool,
            tc.tile_pool(name="x", bufs=1) as xpool,
            tc.tile_pool(name="xb", bufs=1) as xbpool,
            tc.tile_pool(name="wch", bufs=2) as wchpool,
            tc.tile_pool(name="cst", bufs=1) as cpool,
            tc.tile_pool(name="dq", bufs=2) as dqpool,
            tc.tile_pool(name="outp", bufs=3) as opool,
            tc.tile_pool(name="ps", bufs=8, space="PSUM") as pspool,
        ):
            # bias (scalar ring; tiny, out of the input queue's way)
            bias_sb = []
            for ot in range(NO):
                btile = cpool.tile([128, 1], f32, tag=f"bias{ot}", name=f"biassb{ot}")
                nc.scalar.dma_start(
                    out=btile[:], in_=bias[ot * 128 : (ot + 1) * 128, :]
                )
                bias_sb.append(btile)

            # PE warm-up: dummy matmuls so the HAM clock gate opens while
            # inputs stream in. memset on GpSimd (idle; clears its preamble
            # earliest); 8 cold MMs ~= the 3.4us HAM window, short enough
            # not to delay real MMs when the preamble runs late.
            wsc = cpool.tile([128, 512], bf16, tag="wsc", name="wsc")
            nc.gpsimd.memset(wsc[:], 0)
            psc = pspool.tile([128, 512], f32, tag="mm", name="psc")
            NWARM = 10
            for d in range(NWARM):
                nc.tensor.matmul(
                    psc[:], wsc[:, :128], wsc[:],
                    start=(d == 0), stop=(d == NWARM - 1),
                )

            Wp = [
                wpool.tile([128, 2 * O_SH], bf16, tag=f"w{k}", name=f"wt{k}")
                for k in range(NQ)
            ]

            # ---- the ordered input queue (sync ring, FIFO in trace order):
            # x/W chunks interleaved in exact consumption order (quantized
            # pairs first, wd pairs last), then the later t-chunks.

            def wchunk(ci, k0, np_):
                """DMA one W chunk (lh/ba) + its dequant chain.

                DVE per pair: mult 2x (~0.55us) + fp8 add 1x (~1.1us)
                = 1.6us, just under the 1.73us/pair MM consumption rate.
                (A GpSimd fp8->fp16 upconvert was tried and reverted:
                GpSimd CAST is ~3.7us software loop and its shared-port
                lock starves concurrent DVE 2-port ops.)"""
                r0 = k0 - NWD
                lt = wchpool.tile([128, np_, 2 * O_SH], u8, tag="lhc", name=f"lhc{ci}")
                nc.sync.dma_start(out=lt[:], in_=lh[:, r0 : r0 + np_])
                st2 = wchpool.tile([128, np_, O_SH], fp16, tag="scc", name=f"scc{ci}")
                nc.sync.dma_start(out=st2[:], in_=sc[:, r0 : r0 + np_])
                bt = wchpool.tile([128, np_, 2 * O_SH], f8e5, tag="bac", name=f"bac{ci}")
                nc.sync.dma_start(out=bt[:], in_=ba[:, r0 : r0 + np_])
                for j in range(np_):
                    k = k0 + j
                    upf = dqpool.tile([128, 2 * O_SH], fp16, tag="upf", name=f"upf{k}")
                    nc.scalar.activation(
                        upf[:], lt[:, j, :], AF.Copy, bias=-1.0, scale=C16
                    )
                    nc.vector.tensor_tensor(
                        Wp[k][:],
                        upf[:],
                        st2[:, j, :][:, None, :].to_broadcast([128, 2, O_SH]),
                        OP.mult,
                    )
                    nc.vector.tensor_tensor(Wp[k][:], Wp[k][:], bt[:, j, :], OP.add)

            # 2-pair W chunks, each just before the x piece of its pairs —
            # smooth pair-paced supply for both W and x. The last piece
            # belongs to the wd (host-dequantized) pairs.
            x0t = []  # chunk-0 pieces, 2 pairs each
            for gi, (st_, ln) in enumerate(XG):
                if gi < NQ // 2 - 1:
                    wchunk(gi, PSEQ[2 * gi], 2)
                else:
                    nc.sync.dma_start(out=Wp[0][:], in_=wd[:, 0, :])
                    nc.sync.dma_start(out=Wp[1][:], in_=wd[:, 1, :])
                xa = xpool.tile([128, ln, 512], bf16, tag=f"xa{gi}", name=f"xa{gi}")
                x0t.append(xa)
                nc.sync.dma_start(out=xa[:], in_=xt[:, 0, st_ : st_ + ln])

            # later t-chunks: two halves for t1, one whole tile for t2, and
            # t3 halves rotated into t1's slots (WAR deps self-pace: xh0's
            # last reader is mid-t1, long before t3 needs data)
            xh = []
            for hi in range(2):
                xbt = xbpool.tile([128, 16, 512], bf16, tag=f"xh{hi}", name=f"xh{hi}")
                nc.sync.dma_start(out=xbt[:], in_=xt[:, 1, hi * 16 : (hi + 1) * 16])
                xh.append(xbt)
            xc2t = xbpool.tile([128, NI, 512], bf16, tag="xbig", name="xc2")
            nc.sync.dma_start(out=xc2t[:], in_=xt[:, 2])
            x3h = []
            for hi in range(2):
                xbt = xbpool.tile(
                    [128, 16, 512], bf16, tag=f"xh{hi}", name=f"x3h{hi}"
                )
                nc.sync.dma_start(out=xbt[:], in_=xt[:, 3, hi * 16 : (hi + 1) * 16])
                x3h.append(xbt)

            def xsrc(tcn, k, half):
                pos = POS[k + half * NQ]
                if tcn == 0:
                    for gi, (st_, ln) in enumerate(XG):
                        if st_ <= pos < st_ + ln:
                            return x0t[gi][:, pos - st_, :]
                if tcn == 1:
                    return xh[pos // 16][:, pos % 16, :]
                if tcn == 2:
                    return xc2t[:, pos, :]
                return x3h[pos // 16][:, pos % 16, :]

            # ---- main matmuls; stores on ScalarE (activation + bias).
            # t0-t2 pair-major (tolerates streaming x); t3 ot-major so the
            # first 3 stores overlap remaining MMs and only the last store
            # is exposed in the tail.
            def store(p, tcn, ot):
                o_sb = opool.tile([128, 512], bf16, tag="osb", name=f"osb{tcn}_{ot}")
                nc.scalar.activation(
                    o_sb[:], p[:], AF.Identity, bias=bias_sb[ot][:], scale=1.0
                )
                nc.scalar.dma_start(
                    out=out[ot * 128 : (ot + 1) * 128, tcn * 512 : (tcn + 1) * 512],
                    in_=o_sb[:],
                )

            for tcn in range(NT - 1):
                p = [
                    pspool.tile([128, 512], f32, tag="mm", name=f"p{tcn}_{ot}")
                    for ot in range(NO)
                ]
                for si, k in enumerate(PSEQ):
                    for half in range(2):
                        xs = xsrc(tcn, k, half)
                        for ot in range(NO):
                            nc.tensor.matmul(
                                p[ot][:],
                                Wp[k][
                                    :,
                                    half * O_SH + ot * 128 : half * O_SH + (ot + 1) * 128,
                                ],
                                xs,
                                start=(si == 0 and half == 0),
                                stop=(si == NQ - 1 and half == 1),
                            )
                for ot in range(NO):
                    store(p[ot], tcn, ot)

            tcn = NT - 1
            for ot in range(NO):
                p = pspool.tile([128, 512], f32, tag="mm", name=f"p{tcn}_{ot}")
                n = 0
                for k in PSEQ:
                    for half in range(2):
                        nc.tensor.matmul(
                            p[:],
                            Wp[k][
                                :,
                                half * O_SH + ot * 128 : half * O_SH + (ot + 1) * 128,
                            ],
                            xsrc(tcn, k, half),
                            start=(n == 0),
                            stop=(n == 2 * NQ - 1),
                        )
                        n += 1
                store(p, tcn, ot)
    nc.compile()
    return nc


def _pack_rows(a, nblk):
    """[nblk*128, F] -> [128, nblk, F] with blk j, partition p = row j*128+p."""
    f = a.shape[1]
    return np.ascontiguousarray(a.reshape(nblk, 128, f).transpose(1, 0, 2))


def prep_inputs(x, qweight, scales, bias, lora_A, lora_B):
    """Host-side layout prep + sharding. Returns per-core input maps."""
    x2d = np.ascontiguousarray(x.reshape(T, IN_F))
    xtr = x2d.T  # [IN_F, T]
    # i' permutation: even original i first, then odd
    xp = np.concatenate([xtr[0::2], xtr[1::2]], axis=0)
    xb = _pack_rows(xp, NI)  # [128, NI, T]
    xb = np.ascontiguousarray(
        xb.reshape(128, NI, NT, 512).transpose(0, 2, 1, 3)
    )  # [128, NT, NI, 512]
    xtp = np.ascontiguousarray(xb[:, :, X0_ORDER, :]).astype(BF16)

    ap = np.ascontiguousarray(
        np.concatenate([lora_A[:, 0::2], lora_A[:, 1::2]], axis=1)
    ).astype(np.float32)  # [R, IN_F] permuted

    qw2 = qweight.reshape(OUT_F, IPH)  # byte (o, ip) holds i=2ip (lo), 2ip+1 (hi)
    sc2 = scales.reshape(OUT_F, IN_F // 64)

    in_maps = []
    for c in range(NCORES):
        o0, o1 = c * O_SH, (c + 1) * O_SH
        qp = _pack_rows(qw2[o0:o1].T, NQ)  # [128, NQ, O_SH] packed bytes
        lo = (qp & 15).astype(np.float32)
        hi = ((qp >> 4) & 15).astype(np.float32)
        # scale for (ip, o) = scales[o, ip//32] (same for lo and hi nibble)
        st_c = _pack_rows(
            np.repeat(sc2[o0:o1].T, 32, axis=0).astype(np.float32), NQ
        )  # [128, NQ, O_SH]
        ba3 = _pack_rows(
            (ap.T @ (2.0 * lora_B[o0:o1].T)).astype(np.float32), NI
        )  # [128, NI, O_SH]
        ba_pair = np.concatenate(
            [ba3[:, :NQ, :], ba3[:, NQ:, :]], axis=2
        )  # [128, NQ, 2*O_SH]

        # head pairs fully dequantized on host (bf16, ready for matmul)
        wfull = np.concatenate(
            [(lo * C16 - 1.0) * st_c, (hi * C16 - 1.0) * st_c], axis=2
        )
        # device path rounds the scaled value to bf16 then adds the fp8 ba
        # in bf16; mirror roughly by computing in f32 (tolerance is loose)
        wd_c = np.ascontiguousarray(
            (wfull + ba_pair)[:, :NWD, :]
        ).astype(BF16)

        lh_c = np.ascontiguousarray(
            np.concatenate(
                [qp[:, NWD:, :] & 15, (qp[:, NWD:, :] >> 4) & 15], axis=2
            )
        ).astype(np.uint8)  # [128, NLH, 2*O_SH] nibbles
        sc_c = np.ascontiguousarray(st_c[:, NWD:, :]).astype(FP16)
        ba_c = np.ascontiguousarray(ba_pair[:, NWD:, :]).astype(F8E5)
        bias_c = np.ascontiguousarray(bias[o0:o1].reshape(O_SH, 1)).astype(np.float32)
        in_maps.append(
            {
                "xt": xtp,
                "wd": wd_c,
                "lh": lh_c,
                "sc": sc_c,
                "ba": ba_c,
                "bias": bias_c,
            }
        )
    return in_maps


def run(in_maps, trace=False):
    from concourse import bass_utils

    if "nc" not in _cached:
        _cached["nc"] = _build_nc()
    res = bass_utils.run_bass_kernel_spmd(
        _cached["nc"], in_maps, list(range(NCORES)), trace=trace
    )
    return res


def assemble(results):
    full = np.concatenate(
        [np.asarray(r["out"]).astype(np.float32) for r in results], axis=0
    )  # [OUT_F, T]
    return np.ascontiguousarray(full.T).reshape(2, 1024, OUT_F)


def kernel(x, qweight, scales, bias, lora_A, lora_B):
    in_maps = prep_inputs(x, qweight, scales, bias, lora_A, lora_B)
    res = run(in_maps, trace=False)
    return assemble(res.results)
